# revision 17
# baseline (speedup 1.0000x reference)
"""Trainium2 Bass kernel: multi-head attention with relative-position bias.

Problem shapes: x [8, 1024, 768], H=12 heads, d=64.
Strategy: data-parallel over batch (1 element per NeuronCore, 8 cores).
All matmuls in bf16 (f32 PSUM accumulation). Host prep:
  - weights transposed to [C, *] feature-major; q-scale folded into Wq/q_bias
  - relative-position bias gather done as exp(table)[idx] -> bf16 tensor
    [H, Nj, Ni] streamed from HBM and folded into softmax multiplicatively:
    softmax(s + b) = norm(exp(s) * exp(b))   (no row-max needed: |s| < ~10)
Attention computed transposed (sT[j, i]) so softmax sums run along the PE
contraction: PV matmul uses stationary [v | 1], giving the denominator as an
extra output row; normalization via DVE reciprocal + gpsimd partition bcast.
"""
import sys
import numpy as np

sys.path.insert(0, "/opt/trn_rl_repo")

import ml_dtypes

BF16 = ml_dtypes.bfloat16

B, N, C = 8, 1024, 768
H, D = 12, 64
N_CORES = 8
NT = N // 128        # 8 token tiles
CT = C // 128        # 6 feature tiles
OT = 3 * C // 128    # 18 qkv output feature tiles

_cache = {}


def _install_axon_shim():
    """The image's antenv lacks axon_hooks; register the NTFF profile hook so
    run_bass_kernel_spmd(trace=True) works. Safe no-op outside axon."""
    import types

    if "antenv.axon_hooks" not in sys.modules:
        try:
            import antenv
            from trn_agent_boot.trn_boot import _ntff_profile_via_ctypes
        except ImportError:
            return
        mod = types.ModuleType("antenv.axon_hooks")
        _hook = [None]
        mod.set_axon_ntff_profile_hook = lambda h: _hook.__setitem__(0, h)
        mod.get_axon_ntff_profile_hook = lambda: _hook[0]
        sys.modules["antenv.axon_hooks"] = mod
        antenv.axon_hooks = mod
        try:
            mod.set_axon_ntff_profile_hook(
                _ntff_profile_via_ctypes("/opt/axon/libaxon_pjrt.so")
            )
        except Exception:
            pass
    from concourse import bass_utils

    bass_utils.upload_artifacts = lambda tmpdir: tmpdir

    import os
    if os.environ.get("KERNEL_LDW_OPT"):
        orig_run = bass_utils.run_command

        def run_with_ldw(argv, **kwargs):
            argv = [a.replace("--enable-ldw-opt=false", "--enable-ldw-opt=true")
                    for a in argv]
            return orig_run(argv, **kwargs)

        bass_utils.run_command = run_with_ldw


def build_nc():
    from concourse import bacc, mybir, tile

    f32 = mybir.dt.float32
    bf16 = mybir.dt.bfloat16
    AF = mybir.ActivationFunctionType

    nc = bacc.Bacc("TRN2", target_bir_lowering=False, debug=False,
                   num_devices=N_CORES)

    xt_d = nc.dram_tensor("xt", [C, N], bf16, kind="ExternalInput")
    wqkvt_d = nc.dram_tensor("wqkvt", [C, 3 * C], bf16, kind="ExternalInput")
    qkvb_d = nc.dram_tensor("qkvb", [3 * C], f32, kind="ExternalInput")
    vb_d = nc.dram_tensor("vb", [C], f32, kind="ExternalInput")
    wprojt_d = nc.dram_tensor("wprojt", [C, C], bf16, kind="ExternalInput")
    pbias_d = nc.dram_tensor("pbias", [C], f32, kind="ExternalInput")
    ebt_d = nc.dram_tensor("ebt", [H, N, N], bf16, kind="ExternalInput")
    out_d = nc.dram_tensor("out", [N, C], f32, kind="ExternalOutput")

    with tile.TileContext(nc) as tc:
        with (
            tc.tile_pool(name="persist", bufs=1) as persist,
            tc.tile_pool(name="work", bufs=1) as work,
            tc.tile_pool(name="dram", bufs=2, space="DRAM") as dpool,
            tc.tile_pool(name="psum", bufs=1, space="PSUM") as psum,
        ):
            # ---- resident tiles ----
            # q/k feature-major [feature_tile][128, N]; q tiles 0..5, k 6..11
            qk_sb = [persist.tile([128, N], bf16, tag=f"qk{i}", name=f"qk{i}") for i in range(12)]
            # v token-major, 12 groups of (64 vals + 1 one) per token tile
            v_sb = [persist.tile([128, H * 65], bf16, tag=f"v{i}", name=f"v{i}") for i in range(NT)]
            # attention output (pre-proj), feature-major
            ot_sb = [persist.tile([128, N], bf16, tag=f"ot{i}", name=f"ot{i}") for i in range(CT)]
            # small constants
            qkvb_sb = persist.tile([128, OT], f32, tag="qkvb")
            vb_bc = persist.tile([128, C], f32, tag="vb_bc")
            pb_bc = persist.tile([128, C], f32, tag="pb_bc")

            nc.sync.dma_start(qkvb_sb[:], qkvb_d.ap().rearrange("(t p) -> p t", p=128))
            nc.sync.dma_start(vb_bc[:], vb_d.ap().unsqueeze(0).to_broadcast([128, C]))
            nc.sync.dma_start(pb_bc[:], pbias_d.ap().unsqueeze(0).to_broadcast([128, C]))

            wproj_sb = [persist.tile([128, C], bf16, tag=f"wp{i}", name=f"wp{i}") for i in range(CT)]
            for ct in range(CT):
                nc.sync.dma_start(wproj_sb[ct][:], wprojt_d.ap()[ct * 128:(ct + 1) * 128, :])

            xt_sb = [work.tile([128, N], bf16, tag=f"xt{i}", name=f"xt{i}") for i in range(CT)]
            wqkv_sb = [work.tile([128, 3 * C], bf16, tag=f"wq{i}", name=f"wq{i}") for i in range(CT)]
            for ct in range(CT):
                nc.sync.dma_start(xt_sb[ct][:], xt_d.ap()[ct * 128:(ct + 1) * 128, :])
                nc.sync.dma_start(wqkv_sb[ct][:], wqkvt_d.ap()[ct * 128:(ct + 1) * 128, :])

            # ================= Phase A: qkv projection =================
            # q,k feature-major: qkvT[o, n] = sum_c WT[c, o] * xT[c, n]
            def emit_qk_tile(ot):
                ps = psum.tile([128, N], f32, tag="big", name="psa", bufs=2)
                for ct in range(CT):
                    for h2 in range(2):
                        nc.tensor.matmul(
                            ps[:, h2 * 512:(h2 + 1) * 512],
                            wqkv_sb[ct][:, ot * 128:(ot + 1) * 128],
                            xt_sb[ct][:, h2 * 512:(h2 + 1) * 512],
                            start=(ct == 0), stop=(ct == CT - 1),
                            skip_group_check=True,
                        )
                nc.vector.tensor_scalar_add(qk_sb[ot][:], ps[:],
                                            qkvb_sb[:, ot:ot + 1])

            # v token-major: v[n, vd] = sum_c xT[c, n] * WT[c, 2C+vd]
            # NOTE: 384-wide matmul outputs must start at 512-aligned psum
            # offsets (a matmul output may not cross a 2KB PSUM bank).
            def emit_v_tile(nt):
                ps = psum.tile([128, N], f32, tag="big", name="psv", bufs=2)
                for ct in range(CT):
                    for g2 in range(2):
                        nc.tensor.matmul(
                            ps[:, g2 * 512:g2 * 512 + 384],
                            xt_sb[ct][:, nt * 128:(nt + 1) * 128],
                            wqkv_sb[ct][:, 2 * C + g2 * 384:2 * C + (g2 + 1) * 384],
                            start=(ct == 0), stop=(ct == CT - 1),
                            skip_group_check=True,
                        )
                v_view = v_sb[nt][:].rearrange("p (g c) -> p g c", c=65)
                ps_view = (ps[:].rearrange("p (g c) -> p g c", g=2)[:, :, 0:384]
                           .rearrange("p g (h c) -> p g h c", c=64))
                nc.vector.tensor_add(
                    v_view[:, :, 0:64].rearrange("p (g h) c -> p g h c", g=2),
                    ps_view,
                    vb_bc[:].rearrange("p (g h c) -> p g h c", g=2, c=64),
                )
                nc.vector.memset(v_view[:, :, 64:65], 1.0)

            emit_qk_tile(0)
            emit_qk_tile(6)
            for nt in range(NT):
                emit_v_tile(nt)
            for t in range(1, 6):
                emit_qk_tile(t)
                emit_qk_tile(6 + t)

            # ================= Phase B: attention =================
            # Software-pipelined: PV for iteration jc is issued 2 iterations
            # later, so the in-order PE queue never blocks on the
            # exp->mul chain producing pm.
            PV_LAG = 3
            for t in range(6):  # head pairs (2t, 2t+1)
                pv = [[psum.tile([65, 512], f32, tag=f"pv{x}{i}", name=f"pv{x}{i}",
                                 bufs=1)
                       for i in range(2)] for x in range(2)]
                pms = {}

                def emit_pv(jc):
                    for x in range(2):
                        g = 2 * t + x
                        for ic in range(2):
                            nc.tensor.matmul(
                                pv[x][ic][:],
                                v_sb[jc][:, g * 65:(g + 1) * 65],
                                pms[jc][x][:, ic * 512:(ic + 1) * 512],
                                start=(jc == 0), stop=(jc == NT - 1),
                                skip_group_check=True,
                            )

                for jc in range(NT):
                    eb = [work.tile([128, N], bf16, tag="eb", name="eb", bufs=8)
                          for _ in range(2)]
                    for x in range(2):
                        nc.sync.dma_start(eb[x][:],
                                          ebt_d.ap()[2 * t + x,
                                                     jc * 128:(jc + 1) * 128, :])
                    qs = [psum.tile([128, N], f32, tag="big", name="qkps", bufs=2)
                          for _ in range(2)]
                    # interleave heads across row-groups for PE concurrency
                    for ic in range(2):
                        for x in range(2):
                            po = x * 64
                            nc.tensor.matmul(
                                qs[x][:, ic * 512:(ic + 1) * 512],
                                qk_sb[6 + t][po:po + 64, jc * 128:(jc + 1) * 128],
                                qk_sb[t][po:po + 64, ic * 512:(ic + 1) * 512],
                                start=True, stop=True,
                                skip_group_check=True,
                            )
                    pms[jc] = []
                    for x in range(2):
                        pe = work.tile([128, N], bf16, tag="pe", name="pe", bufs=4)
                        nc.scalar.activation(pe[:], qs[x][:], AF.Exp)
                        pm = work.tile([128, N], bf16, tag="pm", name="pm",
                                       bufs=2 * (PV_LAG + 1))
                        nc.vector.tensor_mul(pm[:], pe[:], eb[x][:])
                        pms[jc].append(pm)
                    if jc >= PV_LAG:
                        emit_pv(jc - PV_LAG)
                for jc in range(NT - PV_LAG, NT):
                    emit_pv(jc)
                # ---- normalize: row 64 of pv is the softmax denominator ----
                denom_d = dpool.tile([4, 512], bf16, tag="denom_d", name="denom_d")
                ustage = [[work.tile([65, 512], bf16, tag="ustage", name="ustage", bufs=8)
                           for _ in range(2)] for _ in range(2)]
                for x in range(2):
                    for ic in range(2):
                        r = 2 * x + ic
                        # unnormalized out + denom row -> sbuf (frees psum quickly)
                        nc.vector.tensor_copy(ustage[x][ic][:], pv[x][ic][:])
                        # denominator row -> DRAM scratch (for cross-partition reshape)
                        nc.sync.dma_start(denom_d[r:r + 1, :], ustage[x][ic][64:65, :])
                dstage = work.tile([16, 128], bf16, tag="dstage", name="dstage", bufs=2)
                nc.sync.dma_start(
                    dstage[:], denom_d[:].rearrange("a b -> (a b)").rearrange(
                        "(p c) -> p c", p=16))
                rstage = work.tile([16, 128], bf16, tag="rstage", name="rstage", bufs=2)
                with nc.allow_low_precision("softmax denom recip, 2e-2 gate"):
                    nc.vector.reciprocal(rstage[:], dstage[:])
                rd = dpool.tile([16, 128], bf16, tag="rd", name="rd")
                nc.sync.dma_start(rd[:], rstage[:])
                rd_flat = rd[:].rearrange("p c -> (p c)")
                for x in range(2):
                    for ic in range(2):
                        r = 2 * x + ic
                        rb = work.tile([64, 512], bf16, tag="rb", name="rb", bufs=4)
                        nc.sync.dma_start(
                            rb[:],
                            rd_flat[512 * r:512 * (r + 1)].unsqueeze(0)
                            .to_broadcast([64, 512]),
                        )
                        nc.vector.tensor_mul(
                            ot_sb[t][x * 64:(x + 1) * 64, ic * 512:(ic + 1) * 512],
                            ustage[x][ic][0:64, :],
                            rb[:],
                        )

            # ================= Phase C: output projection =================
            for nt in range(NT):
                ps = psum.tile([128, N], f32, tag="big", name="psc", bufs=2)
                for ct in range(CT):
                    for oc in range(2):
                        nc.tensor.matmul(
                            ps[:, oc * 512:oc * 512 + 384],
                            ot_sb[ct][:, nt * 128:(nt + 1) * 128],
                            wproj_sb[ct][:, oc * 384:(oc + 1) * 384],
                            start=(ct == 0), stop=(ct == CT - 1),
                            skip_group_check=True,
                        )
                osb = work.tile([128, C], f32, tag="osb", name="osb", bufs=3)
                ps_view = ps[:].rearrange("p (g c) -> p g c", g=2)[:, :, 0:384]
                nc.vector.tensor_add(
                    osb[:].rearrange("p (g c) -> p g c", g=2), ps_view,
                    pb_bc[:].rearrange("p (g c) -> p g c", g=2))
                nc.sync.dma_start(out_d.ap()[nt * 128:(nt + 1) * 128, :], osb[:])

    nc.compile()
    return nc


def _get_nc():
    if "nc" not in _cache:
        _install_axon_shim()
        _cache["nc"] = build_nc()
    return _cache["nc"]


def prep_inputs(x, relative_position_index, qkv_weight, q_bias, v_bias,
                proj_weight, proj_bias, rel_pos_bias_table):
    """Host-side layout prep shared by all cores + per-core shards."""
    x = np.asarray(x, np.float32)
    idx = np.asarray(relative_position_index)
    qkv_weight = np.asarray(qkv_weight, np.float32)
    q_bias = np.asarray(q_bias, np.float32)
    v_bias = np.asarray(v_bias, np.float32)
    proj_weight = np.asarray(proj_weight, np.float32)
    proj_bias = np.asarray(proj_bias, np.float32)
    tbl = np.asarray(rel_pos_bias_table, np.float32)

    scale = (C // H) ** (-0.5)
    wq = qkv_weight.copy()
    wq[:C, :] *= scale  # fold softmax scale into q projection
    wqkvt = np.ascontiguousarray(wq.T).astype(BF16)  # [C, 3C]
    qkvb = np.concatenate([q_bias * scale, np.zeros_like(q_bias), v_bias]
                          ).astype(np.float32)
    wprojt = np.ascontiguousarray(proj_weight.T).astype(BF16)  # [C, C]

    # exp(bias) gather: ebt[h, j, i] = exp(table[idx[i, j], h])
    eb = np.exp(tbl)[idx]           # [i, j, H] f32
    ebt = np.ascontiguousarray(eb.transpose(2, 1, 0)).astype(BF16)  # [H, Nj, Ni]

    shared = {
        "wqkvt": wqkvt,
        "qkvb": qkvb,
        "vb": v_bias.astype(np.float32),
        "wprojt": wprojt,
        "pbias": proj_bias.astype(np.float32),
        "ebt": ebt,
    }
    in_maps = []
    for b in range(B):
        m = dict(shared)
        m["xt"] = np.ascontiguousarray(x[b].T).astype(BF16)  # [C, N]
        in_maps.append(m)
    return in_maps


def kernel(**inputs):
    from concourse.bass_utils import run_bass_kernel_spmd

    nc = _get_nc()
    in_maps = prep_inputs(**inputs)
    res = run_bass_kernel_spmd(nc, in_maps, list(range(N_CORES)),
                               trace=False)
    _cache["last_result"] = res
    out = np.stack([res.results[b]["out"] for b in range(B)], axis=0)
    return out.astype(np.float32)


def kernel_profiled(**inputs):
    """Same as kernel() but with NTFF tracing; returns (out, BassKernelResults)."""
    from concourse.bass_utils import run_bass_kernel_spmd

    nc = _get_nc()
    in_maps = prep_inputs(**inputs)
    res = run_bass_kernel_spmd(nc, in_maps, list(range(N_CORES)), trace=True)
    out = np.stack([res.results[b]["out"] for b in range(B)], axis=0)
    return out.astype(np.float32), res


# revision 24
# speedup vs baseline: 1.0048x; 1.0048x over previous
"""Trainium2 Bass kernel: multi-head attention with relative-position bias.

Problem shapes: x [8, 1024, 768], H=12 heads, d=64.
Strategy: data-parallel over batch (1 element per NeuronCore, 8 cores).
All matmuls in bf16 (f32 PSUM accumulation). Host prep:
  - weights transposed to [C, *] feature-major; q-scale folded into Wq/q_bias
  - relative-position bias gather done as exp(table)[idx] -> bf16 tensor
    [H, Nj, Ni] streamed from HBM and folded into softmax multiplicatively:
    softmax(s + b) = norm(exp(s) * exp(b))   (no row-max needed: |s| < ~10)
Attention computed transposed (sT[j, i]) so softmax sums run along the PE
contraction: the PV matmul uses stationary [v | 1], giving the denominator as
an extra psum row for free. Normalization: denominator rows are staged through
DRAM into a [16,128] tile (one batched DVE reciprocal per head pair), then
broadcast back via a stride-0 DRAM-source DMA and applied with one bf16 DVE
multiply per [64,512] tile. Phase B is software-pipelined (PV issued PV_LAG
iterations behind its exp->mul producers) so the in-order PE queue never
stalls on the scalar-engine exp, which is the binding resource (~104us).
"""
import sys
import numpy as np

sys.path.insert(0, "/opt/trn_rl_repo")

import ml_dtypes

BF16 = ml_dtypes.bfloat16

B, N, C = 8, 1024, 768
H, D = 12, 64
N_CORES = 8
NT = N // 128        # 8 token tiles
CT = C // 128        # 6 feature tiles
OT = 3 * C // 128    # 18 qkv output feature tiles

_cache = {}


def _install_axon_shim():
    """The image's antenv lacks axon_hooks; register the NTFF profile hook so
    run_bass_kernel_spmd(trace=True) works. Safe no-op outside axon."""
    import types

    if "antenv.axon_hooks" not in sys.modules:
        try:
            import antenv
            from trn_agent_boot.trn_boot import _ntff_profile_via_ctypes
        except ImportError:
            return
        mod = types.ModuleType("antenv.axon_hooks")
        _hook = [None]
        mod.set_axon_ntff_profile_hook = lambda h: _hook.__setitem__(0, h)
        mod.get_axon_ntff_profile_hook = lambda: _hook[0]
        sys.modules["antenv.axon_hooks"] = mod
        antenv.axon_hooks = mod
        try:
            mod.set_axon_ntff_profile_hook(
                _ntff_profile_via_ctypes("/opt/axon/libaxon_pjrt.so")
            )
        except Exception:
            pass
    from concourse import bass_utils

    bass_utils.upload_artifacts = lambda tmpdir: tmpdir

    import os
    if os.environ.get("KERNEL_LDW_OPT"):
        orig_run = bass_utils.run_command

        def run_with_ldw(argv, **kwargs):
            argv = [a.replace("--enable-ldw-opt=false", "--enable-ldw-opt=true")
                    for a in argv]
            return orig_run(argv, **kwargs)

        bass_utils.run_command = run_with_ldw


def build_nc():
    from concourse import bacc, mybir, tile
    from concourse.tile import add_dep_helper

    f32 = mybir.dt.float32
    bf16 = mybir.dt.bfloat16
    AF = mybir.ActivationFunctionType

    nc = bacc.Bacc("TRN2", target_bir_lowering=False, debug=False,
                   num_devices=N_CORES)

    xt_d = nc.dram_tensor("xt", [C, N], bf16, kind="ExternalInput")
    wqkvt_d = nc.dram_tensor("wqkvt", [C, 3 * C], bf16, kind="ExternalInput")
    qkvb_d = nc.dram_tensor("qkvb", [3 * C], f32, kind="ExternalInput")
    vb_d = nc.dram_tensor("vb", [C], f32, kind="ExternalInput")
    wprojt_d = nc.dram_tensor("wprojt", [C, C], bf16, kind="ExternalInput")
    pbias_d = nc.dram_tensor("pbias", [C], f32, kind="ExternalInput")
    ebt_d = nc.dram_tensor("ebt", [H, N, N], bf16, kind="ExternalInput")
    out_d = nc.dram_tensor("out", [N, C], f32, kind="ExternalOutput")

    with tile.TileContext(nc) as tc:
        with (
            tc.tile_pool(name="persist", bufs=1) as persist,
            tc.tile_pool(name="work", bufs=1) as work,
            tc.tile_pool(name="dram", bufs=2, space="DRAM") as dpool,
            tc.tile_pool(name="psum", bufs=1, space="PSUM") as psum,
        ):
            # ---- resident tiles ----
            # q/k feature-major [feature_tile][128, N]; q tiles 0..5, k 6..11
            qk_sb = [persist.tile([128, N], bf16, tag=f"qk{i}", name=f"qk{i}") for i in range(12)]
            # v token-major, 12 groups of (64 vals + 1 one) per token tile
            v_sb = [persist.tile([128, H * 65], bf16, tag=f"v{i}", name=f"v{i}") for i in range(NT)]
            # attention output (pre-proj), feature-major
            ot_sb = [persist.tile([128, N], bf16, tag=f"ot{i}", name=f"ot{i}") for i in range(CT)]
            # small constants
            qkvb_sb = persist.tile([128, OT], f32, tag="qkvb")
            vb_bc = persist.tile([128, C], f32, tag="vb_bc")
            pb_bc = persist.tile([128, C], f32, tag="pb_bc")

            nc.sync.dma_start(qkvb_sb[:], qkvb_d.ap().rearrange("(t p) -> p t", p=128))
            nc.sync.dma_start(vb_bc[:], vb_d.ap().unsqueeze(0).to_broadcast([128, C]))
            nc.sync.dma_start(pb_bc[:], pbias_d.ap().unsqueeze(0).to_broadcast([128, C]))

            wproj_sb = [persist.tile([128, C], bf16, tag=f"wp{i}", name=f"wp{i}") for i in range(CT)]
            for ct in range(CT):
                nc.sync.dma_start(wproj_sb[ct][:], wprojt_d.ap()[ct * 128:(ct + 1) * 128, :])

            xt_sb = [work.tile([128, N], bf16, tag=f"xt{i}", name=f"xt{i}") for i in range(CT)]
            wqkv_sb = [work.tile([128, 3 * C], bf16, tag=f"wq{i}", name=f"wq{i}") for i in range(CT)]
            xw_dmas = []
            for ct in range(CT):
                xw_dmas.append(nc.sync.dma_start(
                    xt_sb[ct][:], xt_d.ap()[ct * 128:(ct + 1) * 128, :]))
                xw_dmas.append(nc.sync.dma_start(
                    wqkv_sb[ct][:], wqkvt_d.ap()[ct * 128:(ct + 1) * 128, :]))

            # ================= Phase A: qkv projection =================
            # q,k feature-major: qkvT[o, n] = sum_c WT[c, o] * xT[c, n]
            def emit_qk_tile(ot):
                ps = psum.tile([128, N], f32, tag="big", name="psa", bufs=2)
                for ct in range(CT):
                    for h2 in range(2):
                        nc.tensor.matmul(
                            ps[:, h2 * 512:(h2 + 1) * 512],
                            wqkv_sb[ct][:, ot * 128:(ot + 1) * 128],
                            xt_sb[ct][:, h2 * 512:(h2 + 1) * 512],
                            start=(ct == 0), stop=(ct == CT - 1),
                            skip_group_check=True,
                        )
                nc.vector.tensor_scalar_add(qk_sb[ot][:], ps[:],
                                            qkvb_sb[:, ot:ot + 1])

            # v token-major: v[n, vd] = sum_c xT[c, n] * WT[c, 2C+vd]
            # NOTE: 384-wide matmul outputs must start at 512-aligned psum
            # offsets (a matmul output may not cross a 2KB PSUM bank).
            def emit_v_tile(nt):
                ps = psum.tile([128, N], f32, tag="big", name="psv", bufs=2)
                for ct in range(CT):
                    for g2 in range(2):
                        nc.tensor.matmul(
                            ps[:, g2 * 512:g2 * 512 + 384],
                            xt_sb[ct][:, nt * 128:(nt + 1) * 128],
                            wqkv_sb[ct][:, 2 * C + g2 * 384:2 * C + (g2 + 1) * 384],
                            start=(ct == 0), stop=(ct == CT - 1),
                            skip_group_check=True,
                        )
                v_view = v_sb[nt][:].rearrange("p (g c) -> p g c", c=65)
                ps_view = (ps[:].rearrange("p (g c) -> p g c", g=2)[:, :, 0:384]
                           .rearrange("p g (h c) -> p g h c", c=64))
                nc.vector.tensor_add(
                    v_view[:, :, 0:64].rearrange("p (g h) c -> p g h c", g=2),
                    ps_view,
                    vb_bc[:].rearrange("p (g h c) -> p g h c", g=2, c=64),
                )
                nc.vector.memset(v_view[:, :, 64:65], 1.0)

            emit_qk_tile(0)
            emit_qk_tile(6)
            for nt in range(NT):
                emit_v_tile(nt)
            for t in range(1, 6):
                emit_qk_tile(t)
                emit_qk_tile(6 + t)

            # ================= Phase B: attention =================
            # Software-pipelined: PV for iteration jc is issued PV_LAG
            # iterations later, so the in-order PE queue never blocks on the
            # exp->mul chain producing pm.
            PV_LAG = 4
            for t in range(6):  # head pairs (2t, 2t+1)
                pv = [[psum.tile([65, 512], f32, tag=f"pv{x}{i}", name=f"pv{x}{i}",
                                 bufs=1)
                       for i in range(2)] for x in range(2)]
                pms = {}

                def emit_pv_half(jc, x, t=t, pv=pv, pms=pms):
                    g = 2 * t + x
                    for ic in range(2):
                        nc.tensor.matmul(
                            pv[x][ic][:],
                            v_sb[jc][:, g * 65:(g + 1) * 65],
                            pms[jc][x][:, ic * 512:(ic + 1) * 512],
                            start=(jc == 0), stop=(jc == NT - 1),
                            skip_group_check=True,
                        )

                for jc in range(NT):
                    eb = [work.tile([128, N], bf16, tag="eb", name="eb", bufs=8)
                          for _ in range(2)]
                    for x in range(2):
                        nc.sync.dma_start(eb[x][:],
                                          ebt_d.ap()[2 * t + x,
                                                     jc * 128:(jc + 1) * 128, :])
                    pms[jc] = []
                    for x in range(2):
                        po = x * 64
                        qs = psum.tile([128, N], f32, tag="big", name="qkps", bufs=2)
                        for ic in range(2):
                            nc.tensor.matmul(
                                qs[:, ic * 512:(ic + 1) * 512],
                                qk_sb[6 + t][po:po + 64, jc * 128:(jc + 1) * 128],
                                qk_sb[t][po:po + 64, ic * 512:(ic + 1) * 512],
                                start=True, stop=True,
                                skip_group_check=True,
                            )
                        pe = work.tile([128, N], bf16, tag="pe", name="pe", bufs=6)
                        nc.scalar.activation(pe[:], qs[:], AF.Exp)
                        pm = work.tile([128, N], bf16, tag="pm", name="pm", bufs=14)
                        nc.vector.tensor_mul(pm[:], pe[:], eb[x][:])
                        pms[jc].append(pm)
                        if jc >= PV_LAG:
                            emit_pv_half(jc - PV_LAG, x)
                for jc in range(NT - PV_LAG, NT):
                    emit_pv_half(jc, 0)
                    emit_pv_half(jc, 1)
                # ---- normalize: row 64 of pv is the softmax denominator ----
                denom_d = dpool.tile([4, 512], bf16, tag="denom_d", name="denom_d")
                ustage = [[work.tile([65, 512], bf16, tag="ustage", name="ustage", bufs=8)
                           for _ in range(2)] for _ in range(2)]
                for x in range(2):
                    for ic in range(2):
                        r = 2 * x + ic
                        nc.vector.tensor_copy(ustage[x][ic][:], pv[x][ic][:])
                        nc.sync.dma_start(denom_d[r:r + 1, :], ustage[x][ic][64:65, :])
                dstage = work.tile([16, 128], bf16, tag="dstage", name="dstage", bufs=2)
                nc.sync.dma_start(
                    dstage[:], denom_d[:].rearrange("a b -> (a b)").rearrange(
                        "(p c) -> p c", p=16))
                rstage = work.tile([16, 128], bf16, tag="rstage", name="rstage", bufs=2)
                with nc.allow_low_precision("softmax denom recip, 2e-2 gate"):
                    nc.vector.reciprocal(rstage[:], dstage[:])
                rd = dpool.tile([16, 128], bf16, tag="rd", name="rd")
                nc.sync.dma_start(rd[:], rstage[:])
                rd_flat = rd[:].rearrange("p c -> (p c)")
                for x in range(2):
                    for ic in range(2):
                        r = 2 * x + ic
                        rb = work.tile([64, 512], bf16, tag="rb", name="rb", bufs=4)
                        nc.sync.dma_start(
                            rb[:],
                            rd_flat[512 * r:512 * (r + 1)].unsqueeze(0)
                            .to_broadcast([64, 512]),
                        )
                        nc.vector.tensor_mul(
                            ot_sb[t][x * 64:(x + 1) * 64, ic * 512:(ic + 1) * 512],
                            ustage[x][ic][0:64, :],
                            rb[:],
                        )

            # ================= Phase C: output projection =================
            for nt in range(NT):
                ps = psum.tile([128, N], f32, tag="big", name="psc", bufs=2)
                for ct in range(CT):
                    for oc in range(2):
                        nc.tensor.matmul(
                            ps[:, oc * 512:oc * 512 + 384],
                            ot_sb[ct][:, nt * 128:(nt + 1) * 128],
                            wproj_sb[ct][:, oc * 384:(oc + 1) * 384],
                            start=(ct == 0), stop=(ct == CT - 1),
                            skip_group_check=True,
                        )
                osb = work.tile([128, C], f32, tag="osb", name="osb", bufs=3)
                ps_view = ps[:].rearrange("p (g c) -> p g c", g=2)[:, :, 0:384]
                nc.vector.tensor_add(
                    osb[:].rearrange("p (g c) -> p g c", g=2), ps_view,
                    pb_bc[:].rearrange("p (g c) -> p g c", g=2))
                nc.sync.dma_start(out_d.ap()[nt * 128:(nt + 1) * 128, :], osb[:])

    nc.compile()
    return nc


def _get_nc():
    if "nc" not in _cache:
        _install_axon_shim()
        _cache["nc"] = build_nc()
    return _cache["nc"]


def prep_inputs(x, relative_position_index, qkv_weight, q_bias, v_bias,
                proj_weight, proj_bias, rel_pos_bias_table):
    """Host-side layout prep shared by all cores + per-core shards."""
    x = np.asarray(x, np.float32)
    idx = np.asarray(relative_position_index)
    qkv_weight = np.asarray(qkv_weight, np.float32)
    q_bias = np.asarray(q_bias, np.float32)
    v_bias = np.asarray(v_bias, np.float32)
    proj_weight = np.asarray(proj_weight, np.float32)
    proj_bias = np.asarray(proj_bias, np.float32)
    tbl = np.asarray(rel_pos_bias_table, np.float32)

    scale = (C // H) ** (-0.5)
    wq = qkv_weight.copy()
    wq[:C, :] *= scale  # fold softmax scale into q projection
    wqkvt = np.ascontiguousarray(wq.T).astype(BF16)  # [C, 3C]
    qkvb = np.concatenate([q_bias * scale, np.zeros_like(q_bias), v_bias]
                          ).astype(np.float32)
    wprojt = np.ascontiguousarray(proj_weight.T).astype(BF16)  # [C, C]

    # exp(bias) gather: ebt[h, j, i] = exp(table[idx[i, j], h])
    eb = np.exp(tbl)[idx]           # [i, j, H] f32
    ebt = np.ascontiguousarray(eb.transpose(2, 1, 0)).astype(BF16)  # [H, Nj, Ni]

    shared = {
        "wqkvt": wqkvt,
        "qkvb": qkvb,
        "vb": v_bias.astype(np.float32),
        "wprojt": wprojt,
        "pbias": proj_bias.astype(np.float32),
        "ebt": ebt,
    }
    in_maps = []
    for b in range(B):
        m = dict(shared)
        m["xt"] = np.ascontiguousarray(x[b].T).astype(BF16)  # [C, N]
        in_maps.append(m)
    return in_maps


def kernel(**inputs):
    from concourse.bass_utils import run_bass_kernel_spmd

    nc = _get_nc()
    in_maps = prep_inputs(**inputs)
    res = run_bass_kernel_spmd(nc, in_maps, list(range(N_CORES)),
                               trace=False)
    _cache["last_result"] = res
    out = np.stack([res.results[b]["out"] for b in range(B)], axis=0)
    return out.astype(np.float32)


def kernel_profiled(**inputs):
    """Same as kernel() but with NTFF tracing; returns (out, BassKernelResults)."""
    from concourse.bass_utils import run_bass_kernel_spmd

    nc = _get_nc()
    in_maps = prep_inputs(**inputs)
    res = run_bass_kernel_spmd(nc, in_maps, list(range(N_CORES)), trace=True)
    out = np.stack([res.results[b]["out"] for b in range(B)], axis=0)
    return out.astype(np.float32), res


# revision 25
# speedup vs baseline: 1.0938x; 1.0885x over previous
"""Trainium2 Bass kernel: multi-head attention with relative-position bias.

Problem shapes: x [8, 1024, 768], H=12 heads, d=64.
Strategy: data-parallel over batch (1 element per NeuronCore, 8 cores).
All matmuls in bf16 (f32 PSUM accumulation). Host prep:
  - weights transposed to [C, *] feature-major; q-scale folded into Wq/q_bias
  - relative-position bias gather done as exp(table)[idx] -> bf16 tensor
    [H, Nj, Ni] streamed from HBM and folded into softmax multiplicatively:
    softmax(s + b) = norm(exp(s) * exp(b))   (no row-max needed: |s| < ~10)
Attention computed transposed (sT[j, i]) so softmax sums run along the PE
contraction: the PV matmul uses stationary [v | 1], giving the denominator as
an extra psum row for free. Normalization: denominator rows are staged through
DRAM into a [16,128] tile (one batched DVE reciprocal per head pair), then
broadcast back via a stride-0 DRAM-source DMA and applied with one bf16 DVE
multiply per [64,512] tile. Phase B is software-pipelined (PV issued PV_LAG
iterations behind its exp->mul producers) so the in-order PE queue never
stalls on the scalar-engine exp, which is the binding resource (~104us).
"""
import sys
import numpy as np

sys.path.insert(0, "/opt/trn_rl_repo")

import ml_dtypes

BF16 = ml_dtypes.bfloat16

B, N, C = 8, 1024, 768
H, D = 12, 64
N_CORES = 8
NT = N // 128        # 8 token tiles
CT = C // 128        # 6 feature tiles
OT = 3 * C // 128    # 18 qkv output feature tiles

_cache = {}


def _install_axon_shim():
    """The image's antenv lacks axon_hooks; register the NTFF profile hook so
    run_bass_kernel_spmd(trace=True) works. Safe no-op outside axon."""
    import types

    if "antenv.axon_hooks" not in sys.modules:
        try:
            import antenv
            from trn_agent_boot.trn_boot import _ntff_profile_via_ctypes
        except ImportError:
            return
        mod = types.ModuleType("antenv.axon_hooks")
        _hook = [None]
        mod.set_axon_ntff_profile_hook = lambda h: _hook.__setitem__(0, h)
        mod.get_axon_ntff_profile_hook = lambda: _hook[0]
        sys.modules["antenv.axon_hooks"] = mod
        antenv.axon_hooks = mod
        try:
            mod.set_axon_ntff_profile_hook(
                _ntff_profile_via_ctypes("/opt/axon/libaxon_pjrt.so")
            )
        except Exception:
            pass
    from concourse import bass_utils

    bass_utils.upload_artifacts = lambda tmpdir: tmpdir

    import os
    if os.environ.get("KERNEL_LDW_OPT"):
        orig_run = bass_utils.run_command

        def run_with_ldw(argv, **kwargs):
            argv = [a.replace("--enable-ldw-opt=false", "--enable-ldw-opt=true")
                    for a in argv]
            return orig_run(argv, **kwargs)

        bass_utils.run_command = run_with_ldw


def build_nc():
    from concourse import bacc, mybir, tile
    from concourse.tile import add_dep_helper

    f32 = mybir.dt.float32
    bf16 = mybir.dt.bfloat16
    AF = mybir.ActivationFunctionType

    nc = bacc.Bacc("TRN2", target_bir_lowering=False, debug=False,
                   num_devices=N_CORES)

    xt_d = nc.dram_tensor("xt", [C, N], bf16, kind="ExternalInput")
    wqkvt_d = nc.dram_tensor("wqkvt", [C, 3 * C], bf16, kind="ExternalInput")
    qkvb_d = nc.dram_tensor("qkvb", [3 * C], f32, kind="ExternalInput")
    vb_d = nc.dram_tensor("vb", [C], f32, kind="ExternalInput")
    wprojt_d = nc.dram_tensor("wprojt", [C, C], bf16, kind="ExternalInput")
    pbias_d = nc.dram_tensor("pbias", [C], f32, kind="ExternalInput")
    ebt_d = nc.dram_tensor("ebt", [H, N, N], bf16, kind="ExternalInput")
    out_d = nc.dram_tensor("out", [N, C], f32, kind="ExternalOutput")

    with tile.TileContext(nc) as tc:
        with (
            tc.tile_pool(name="persist", bufs=1) as persist,
            tc.tile_pool(name="work", bufs=1) as work,
            tc.tile_pool(name="dram", bufs=2, space="DRAM") as dpool,
            tc.tile_pool(name="psum", bufs=1, space="PSUM") as psum,
        ):
            # ---- resident tiles ----
            # q/k feature-major [feature_tile][128, N]; q tiles 0..5, k 6..11
            qk_sb = [persist.tile([128, N], bf16, tag=f"qk{i}", name=f"qk{i}") for i in range(12)]
            # v token-major, 12 groups of (64 vals + 1 one) per token tile
            v_sb = [persist.tile([128, H * 65], bf16, tag=f"v{i}", name=f"v{i}") for i in range(NT)]
            # attention output (pre-proj), feature-major
            ot_sb = [persist.tile([128, N], bf16, tag=f"ot{i}", name=f"ot{i}") for i in range(CT)]
            # small constants
            qkvb_sb = persist.tile([128, OT], f32, tag="qkvb")
            vb_bc = persist.tile([128, C], f32, tag="vb_bc")
            pb_bc = persist.tile([128, C], f32, tag="pb_bc")

            nc.sync.dma_start(qkvb_sb[:], qkvb_d.ap().rearrange("(t p) -> p t", p=128))
            nc.sync.dma_start(vb_bc[:], vb_d.ap().unsqueeze(0).to_broadcast([128, C]))
            nc.sync.dma_start(pb_bc[:], pbias_d.ap().unsqueeze(0).to_broadcast([128, C]))

            wproj_sb = [persist.tile([128, C], bf16, tag=f"wp{i}", name=f"wp{i}") for i in range(CT)]
            for ct in range(CT):
                nc.sync.dma_start(wproj_sb[ct][:], wprojt_d.ap()[ct * 128:(ct + 1) * 128, :])

            xt_sb = [work.tile([128, N], bf16, tag=f"xt{i}", name=f"xt{i}") for i in range(CT)]
            wqkv_sb = [work.tile([128, 3 * C], bf16, tag=f"wq{i}", name=f"wq{i}") for i in range(CT)]
            xw_dmas = []
            for ct in range(CT):
                xw_dmas.append(nc.sync.dma_start(
                    xt_sb[ct][:], xt_d.ap()[ct * 128:(ct + 1) * 128, :]))
                xw_dmas.append(nc.sync.dma_start(
                    wqkv_sb[ct][:], wqkvt_d.ap()[ct * 128:(ct + 1) * 128, :]))

            # ================= Phase A: qkv projection =================
            # q,k feature-major: qkvT[o, n] = sum_c WT[c, o] * xT[c, n]
            def emit_qk_tile(ot):
                ps = psum.tile([128, N], f32, tag="big", name="psa", bufs=2)
                for ct in range(CT):
                    for h2 in range(2):
                        nc.tensor.matmul(
                            ps[:, h2 * 512:(h2 + 1) * 512],
                            wqkv_sb[ct][:, ot * 128:(ot + 1) * 128],
                            xt_sb[ct][:, h2 * 512:(h2 + 1) * 512],
                            start=(ct == 0), stop=(ct == CT - 1),
                            skip_group_check=True,
                        )
                nc.vector.tensor_scalar_add(qk_sb[ot][:], ps[:],
                                            qkvb_sb[:, ot:ot + 1])

            # v token-major: v[n, vd] = sum_c xT[c, n] * WT[c, 2C+vd]
            # NOTE: 384-wide matmul outputs must start at 512-aligned psum
            # offsets (a matmul output may not cross a 2KB PSUM bank).
            def emit_v_tile(nt):
                ps = psum.tile([128, N], f32, tag="big", name="psv", bufs=2)
                for ct in range(CT):
                    for g2 in range(2):
                        nc.tensor.matmul(
                            ps[:, g2 * 512:g2 * 512 + 384],
                            xt_sb[ct][:, nt * 128:(nt + 1) * 128],
                            wqkv_sb[ct][:, 2 * C + g2 * 384:2 * C + (g2 + 1) * 384],
                            start=(ct == 0), stop=(ct == CT - 1),
                            skip_group_check=True,
                        )
                v_view = v_sb[nt][:].rearrange("p (g c) -> p g c", c=65)
                ps_view = (ps[:].rearrange("p (g c) -> p g c", g=2)[:, :, 0:384]
                           .rearrange("p g (h c) -> p g h c", c=64))
                nc.vector.tensor_add(
                    v_view[:, :, 0:64].rearrange("p (g h) c -> p g h c", g=2),
                    ps_view,
                    vb_bc[:].rearrange("p (g h c) -> p g h c", g=2, c=64),
                )
                nc.vector.memset(v_view[:, :, 64:65], 1.0)

            emit_qk_tile(0)
            emit_qk_tile(6)
            for nt in range(NT):
                emit_v_tile(nt)
            for t in range(1, 6):
                emit_qk_tile(t)
                emit_qk_tile(6 + t)

            # ================= Phase B: attention =================
            # Software-pipelined: PV for iteration jc is issued PV_LAG
            # iterations later, so the in-order PE queue never blocks on the
            # exp->mul chain producing pm.
            PV_LAG = 4
            for t in range(6):  # head pairs (2t, 2t+1)
                pv = [[psum.tile([65, 512], f32, tag=f"pv{x}{i}", name=f"pv{x}{i}",
                                 bufs=1)
                       for i in range(2)] for x in range(2)]
                pms = {}

                def emit_pv_half(jc, x, t=t, pv=pv, pms=pms):
                    g = 2 * t + x
                    for ic in range(2):
                        nc.tensor.matmul(
                            pv[x][ic][:],
                            v_sb[jc][:, g * 65:(g + 1) * 65],
                            pms[jc][x][:, ic * 512:(ic + 1) * 512],
                            start=(jc == 0), stop=(jc == NT - 1),
                            skip_group_check=True,
                        )

                for jc in range(NT):
                    eb = [work.tile([128, N], bf16, tag="eb", name="eb", bufs=12)
                          for _ in range(2)]
                    for x in range(2):
                        nc.sync.dma_start(eb[x][:],
                                          ebt_d.ap()[2 * t + x,
                                                     jc * 128:(jc + 1) * 128, :])
                    pms[jc] = []
                    for x in range(2):
                        po = x * 64
                        qs = psum.tile([128, N], f32, tag="big", name="qkps", bufs=2)
                        for ic in range(2):
                            nc.tensor.matmul(
                                qs[:, ic * 512:(ic + 1) * 512],
                                qk_sb[6 + t][po:po + 64, jc * 128:(jc + 1) * 128],
                                qk_sb[t][po:po + 64, ic * 512:(ic + 1) * 512],
                                start=True, stop=True,
                                skip_group_check=True,
                            )
                        pe = work.tile([128, N], bf16, tag="pe", name="pe", bufs=6)
                        nc.scalar.activation(pe[:], qs[:], AF.Exp)
                        pm = work.tile([128, N], bf16, tag="pm", name="pm", bufs=14)
                        nc.vector.tensor_mul(pm[:], pe[:], eb[x][:])
                        pms[jc].append(pm)
                        if jc >= PV_LAG:
                            emit_pv_half(jc - PV_LAG, x)
                for jc in range(NT - PV_LAG, NT):
                    emit_pv_half(jc, 0)
                    emit_pv_half(jc, 1)
                # ---- normalize: row 64 of pv is the softmax denominator ----
                denom_d = dpool.tile([4, 512], bf16, tag="denom_d", name="denom_d")
                ustage = [[work.tile([65, 512], bf16, tag="ustage", name="ustage", bufs=8)
                           for _ in range(2)] for _ in range(2)]
                for x in range(2):
                    for ic in range(2):
                        r = 2 * x + ic
                        nc.vector.tensor_copy(ustage[x][ic][:], pv[x][ic][:])
                        nc.sync.dma_start(denom_d[r:r + 1, :], ustage[x][ic][64:65, :])
                dstage = work.tile([16, 128], bf16, tag="dstage", name="dstage", bufs=2)
                nc.sync.dma_start(
                    dstage[:], denom_d[:].rearrange("a b -> (a b)").rearrange(
                        "(p c) -> p c", p=16))
                rstage = work.tile([16, 128], bf16, tag="rstage", name="rstage", bufs=2)
                with nc.allow_low_precision("softmax denom recip, 2e-2 gate"):
                    nc.vector.reciprocal(rstage[:], dstage[:])
                rd = dpool.tile([16, 128], bf16, tag="rd", name="rd")
                nc.sync.dma_start(rd[:], rstage[:])
                rd_flat = rd[:].rearrange("p c -> (p c)")
                for x in range(2):
                    for ic in range(2):
                        r = 2 * x + ic
                        rb = work.tile([64, 512], bf16, tag="rb", name="rb", bufs=4)
                        nc.sync.dma_start(
                            rb[:],
                            rd_flat[512 * r:512 * (r + 1)].unsqueeze(0)
                            .to_broadcast([64, 512]),
                        )
                        nc.vector.tensor_mul(
                            ot_sb[t][x * 64:(x + 1) * 64, ic * 512:(ic + 1) * 512],
                            ustage[x][ic][0:64, :],
                            rb[:],
                        )

            # ================= Phase C: output projection =================
            for nt in range(NT):
                ps = psum.tile([128, N], f32, tag="big", name="psc", bufs=2)
                for ct in range(CT):
                    for oc in range(2):
                        nc.tensor.matmul(
                            ps[:, oc * 512:oc * 512 + 384],
                            ot_sb[ct][:, nt * 128:(nt + 1) * 128],
                            wproj_sb[ct][:, oc * 384:(oc + 1) * 384],
                            start=(ct == 0), stop=(ct == CT - 1),
                            skip_group_check=True,
                        )
                osb = work.tile([128, C], f32, tag="osb", name="osb", bufs=3)
                ps_view = ps[:].rearrange("p (g c) -> p g c", g=2)[:, :, 0:384]
                nc.vector.tensor_add(
                    osb[:].rearrange("p (g c) -> p g c", g=2), ps_view,
                    pb_bc[:].rearrange("p (g c) -> p g c", g=2))
                nc.sync.dma_start(out_d.ap()[nt * 128:(nt + 1) * 128, :], osb[:])

    nc.compile()
    return nc


def _get_nc():
    if "nc" not in _cache:
        _install_axon_shim()
        _cache["nc"] = build_nc()
    return _cache["nc"]


def prep_inputs(x, relative_position_index, qkv_weight, q_bias, v_bias,
                proj_weight, proj_bias, rel_pos_bias_table):
    """Host-side layout prep shared by all cores + per-core shards."""
    x = np.asarray(x, np.float32)
    idx = np.asarray(relative_position_index)
    qkv_weight = np.asarray(qkv_weight, np.float32)
    q_bias = np.asarray(q_bias, np.float32)
    v_bias = np.asarray(v_bias, np.float32)
    proj_weight = np.asarray(proj_weight, np.float32)
    proj_bias = np.asarray(proj_bias, np.float32)
    tbl = np.asarray(rel_pos_bias_table, np.float32)

    scale = (C // H) ** (-0.5)
    wq = qkv_weight.copy()
    wq[:C, :] *= scale  # fold softmax scale into q projection
    wqkvt = np.ascontiguousarray(wq.T).astype(BF16)  # [C, 3C]
    qkvb = np.concatenate([q_bias * scale, np.zeros_like(q_bias), v_bias]
                          ).astype(np.float32)
    wprojt = np.ascontiguousarray(proj_weight.T).astype(BF16)  # [C, C]

    # exp(bias) gather: ebt[h, j, i] = exp(table[idx[i, j], h])
    eb = np.exp(tbl)[idx]           # [i, j, H] f32
    ebt = np.ascontiguousarray(eb.transpose(2, 1, 0)).astype(BF16)  # [H, Nj, Ni]

    shared = {
        "wqkvt": wqkvt,
        "qkvb": qkvb,
        "vb": v_bias.astype(np.float32),
        "wprojt": wprojt,
        "pbias": proj_bias.astype(np.float32),
        "ebt": ebt,
    }
    in_maps = []
    for b in range(B):
        m = dict(shared)
        m["xt"] = np.ascontiguousarray(x[b].T).astype(BF16)  # [C, N]
        in_maps.append(m)
    return in_maps


def kernel(**inputs):
    from concourse.bass_utils import run_bass_kernel_spmd

    nc = _get_nc()
    in_maps = prep_inputs(**inputs)
    res = run_bass_kernel_spmd(nc, in_maps, list(range(N_CORES)),
                               trace=False)
    _cache["last_result"] = res
    out = np.stack([res.results[b]["out"] for b in range(B)], axis=0)
    return out.astype(np.float32)


def kernel_profiled(**inputs):
    """Same as kernel() but with NTFF tracing; returns (out, BassKernelResults)."""
    from concourse.bass_utils import run_bass_kernel_spmd

    nc = _get_nc()
    in_maps = prep_inputs(**inputs)
    res = run_bass_kernel_spmd(nc, in_maps, list(range(N_CORES)), trace=True)
    out = np.stack([res.results[b]["out"] for b in range(B)], axis=0)
    return out.astype(np.float32), res
